# revision 24
# baseline (speedup 1.0000x reference)
"""Trainium2 Bass kernel for HierarchicalBG embedding lookup (bicubic
pano-grid sampling + tiny MLP), data-parallel over rays on 8 NeuronCores.

Key facts exploited:
- Level-2 grid weight clip(1-mip,0,1) == 0 identically (mip >= 1): the 256MB
  bg_mat2 is never read.
- Level-1 weight clip(2-mip,0,1) == 0 for ~82% of rays (a pure function of
  saSample, known on host). Host sorts level-1-needing rays into the first
  l1_chunks chunks of each core; level-1 gathers/weighting are only issued
  there (graceful degradation: any overflow hits s1 -> 0 rays).
- Remaining rays are sorted by level-0 table position so consecutive
  gathers hit nearby HBM rows.
- Grids are re-laid out on host into a 4-y-tap expanded table so one ray's
  4x4x8 bicubic footprint is one contiguous 512B run -> one indirect-DMA
  descriptor per (ray, level).
- Per-ray tap weights (wx x wy outer product, mip weight folded in) applied
  on DVE in ray-major layout; tap reduction on DVE; 8->128->3 MLP on PE after
  a 128x128 PE transpose.
"""

import numpy as np
from contextlib import ExitStack

PI = float(np.pi)
RANK = 8
P = 128

# cubic weights as polys in t = frac coordinate, coeffs (d, c, b, a) for
# w = ((d*t + c)*t + b)*t + a   (A = -0.75, matches reference _cubic_weights)
CUBIC = [
    (-0.75, 1.50, -0.75, 0.0),   # c2(1+t)
    (1.25, -2.25, 0.0, 1.0),     # c1(t)
    (-1.25, 1.50, 0.75, 0.0),    # c1(1-t)
    (0.75, -0.75, 0.0, 0.0),     # c2(2-t)
]

FULL_CFG = dict(
    nrc=32768,
    dims=((512, 1024), (1024, 2048)),
    res_mip=2048,
    nr_chunk=32,
    l1_chunks=2,
    l1_cols=48,
    num_devices=8,
)


def _expand_table(img):
    """[C, H, W] -> [(H+1)*W + 8, 32]: entry (R, x) = rows R-2..R+1 at col x,
    channels innermost, zero outside the image; +2 entry global pad."""
    C, H, W = img.shape
    imgT = np.ascontiguousarray(np.asarray(img, np.float32).transpose(1, 2, 0))
    ex = np.zeros((H + 1, W, 4, C), np.float32)
    for j in range(4):
        lo = max(0, 2 - j)
        hi = min(H, H + 1 - j)
        ex[lo:hi + 1, :, j, :] = imgT[lo - 2 + j:hi - 1 + j, :, :]
    flat = np.zeros(((H + 1) * W + 8, 4 * C), np.float32)
    flat[2:2 + (H + 1) * W] = ex.reshape(-1, 4 * C)
    return flat


def build_nc(cfg):
    import concourse.bass as bass
    import concourse.tile as tile
    from concourse import bacc, mybir

    f32 = mybir.dt.float32
    i32 = mybir.dt.int32
    Alu = mybir.AluOpType
    Act = mybir.ActivationFunctionType

    nrc, dims, NRCH = cfg["nrc"], cfg["dims"], cfg["nr_chunk"]
    NRP = nrc // P
    NCHUNK = NRP // NRCH
    L1CH = cfg["l1_chunks"]
    CL = L1CH * NRCH                       # columns with level-1 geometry
    L1C = cfg["l1_cols"]                   # columns with level-1 gathers
    GR = min(16, NRP)
    n_ent = [(h + 1) * w + 8 for (h, w) in dims]
    saTexel = 4.0 * PI / (6.0 * cfg["res_mip"] ** 2)
    MIPC1 = 1.0 / (2.0 * np.log(2.0))
    MIPC2 = -float(np.log(saTexel)) * MIPC1

    nc = bacc.Bacc("TRN2", target_bir_lowering=False, debug=False,
                   num_devices=cfg["num_devices"],
                   dynamic_dma_scratch_size=cfg.get("dma_scratch", 32768))
    vd = nc.dram_tensor("vdT", [3, P, NRP], f32, kind="ExternalInput").ap()
    sa = nc.dram_tensor("sa", [P, NRP], f32, kind="ExternalInput").ap()
    ex = [nc.dram_tensor(f"ex{l}", [n_ent[l], 4 * RANK], f32,
                         kind="ExternalInput").ap() for l in range(2)]
    w1r = nc.dram_tensor("w1big", [P, 16 * P], f32, kind="ExternalInput").ap()
    w2t = nc.dram_tensor("w2t", [P, 3], f32, kind="ExternalInput").ap()
    out_d = nc.dram_tensor("out", [NRP // GR, 3, GR * P], f32,
                           kind="ExternalOutput").ap()

    def cap(tile_obj, offset, ap_list):
        """custom AP over a tile's underlying tensor (element units)."""
        base = tile_obj[:]
        return bass.AP(base.tensor, base.offset + offset, ap_list)

    with tile.TileContext(nc) as tc, ExitStack() as ctx:
        from concourse.masks import make_identity
        cpool = ctx.enter_context(tc.tile_pool(name="const", bufs=1))
        geom = ctx.enter_context(tc.tile_pool(name="geom", bufs=1))
        gpool = ctx.enter_context(tc.tile_pool(name="gath", bufs=2))
        mpool = ctx.enter_context(tc.tile_pool(name="mlp", bufs=2))
        pp = ctx.enter_context(tc.tile_pool(name="ps", bufs=2, space="PSUM"))
        tpool = ctx.enter_context(tc.tile_pool(name="tmp", bufs=12))
        dve, act = nc.vector, nc.scalar

        _tag = [0]

        def t(shape=None, dt=f32):
            _tag[0] += 1
            return tpool.tile([P, NRP] if shape is None else shape, dt,
                              name=f"g{_tag[0]}", tag="tmp")

        def pt(nm, shape=None, dt=f32):
            return geom.tile([P, NRP] if shape is None else shape, dt,
                             name=nm, tag=nm)

        _cb = {}

        def cbias(val):
            if val not in _cb:
                ct = cpool.tile([P, 1], f32, name=f"cb{len(_cb)}",
                                tag=f"cb{len(_cb)}")
                nc.vector.memset(ct[:], float(val))
                _cb[val] = ct
            return _cb[val][:]

        ident = cpool.tile([P, P], f32, name="ident", tag="ident")
        make_identity(nc, ident[:])
        w1_sb = cpool.tile([P, 16 * P], f32, name="w1c_", tag="w1")
        nc.sync.dma_start(w1_sb[:], w1r[:, :])
        w2_sb = cpool.tile([P, 3], f32, name="w2c_", tag="w2")
        nc.sync.dma_start(w2_sb[:], w2t[:, :])

        xt, yt, zt, sat = pt("xt"), pt("yt"), pt("zt"), pt("sat")
        nc.sync.dma_start(xt[:], vd[0])
        nc.sync.dma_start(yt[:], vd[1])
        nc.sync.dma_start(zt[:], vd[2])
        nc.sync.dma_start(sat[:], sa[:, :])

        # ---- gx*pi = atan2(x,z) via range-reduced arctan ----
        ax = t(); act.activation(ax[:], xt[:], Act.Abs, bias=cbias(0.0))
        az = t(); act.activation(az[:], zt[:], Act.Abs, bias=cbias(0.0))
        mn = t(); dve.tensor_tensor(out=mn[:], in0=ax[:], in1=az[:], op=Alu.min)
        mx = t(); dve.tensor_tensor(out=mx[:], in0=ax[:], in1=az[:], op=Alu.max)
        dve.tensor_scalar(out=mx[:], in0=mx[:], scalar1=1e-38, scalar2=None,
                          op0=Alu.max)
        rmx = t(); dve.reciprocal(rmx[:], mx[:])
        r = t(); dve.tensor_tensor(out=r[:], in0=mn[:], in1=rmx[:], op=Alu.mult)
        a = t(); act.activation(a[:], r[:], Act.Arctan, bias=cbias(0.0))
        swp = t(); dve.tensor_tensor(out=swp[:], in0=ax[:], in1=az[:],
                                     op=Alu.is_gt)
        f1 = t(); dve.tensor_scalar(out=f1[:], in0=swp[:], scalar1=-2.0,
                                    scalar2=1.0, op0=Alu.mult, op1=Alu.add)
        a1 = t(); dve.tensor_tensor(out=a1[:], in0=a[:], in1=f1[:], op=Alu.mult)
        dve.scalar_tensor_tensor(out=a1[:], in0=swp[:], scalar=PI / 2,
                                 in1=a1[:], op0=Alu.mult, op1=Alu.add)
        szlt = t(); dve.tensor_scalar(out=szlt[:], in0=zt[:], scalar1=0.0,
                                      scalar2=None, op0=Alu.is_lt)
        f2 = t(); dve.tensor_scalar(out=f2[:], in0=szlt[:], scalar1=-2.0,
                                    scalar2=1.0, op0=Alu.mult, op1=Alu.add)
        a2 = t(); dve.tensor_tensor(out=a2[:], in0=a1[:], in1=f2[:],
                                    op=Alu.mult)
        dve.scalar_tensor_tensor(out=a2[:], in0=szlt[:], scalar=PI,
                                 in1=a2[:], op0=Alu.mult, op1=Alu.add)
        sgx = t(); dve.tensor_scalar(out=sgx[:], in0=xt[:], scalar1=0.0,
                                     scalar2=None, op0=Alu.is_ge)
        sgx2 = t(); dve.tensor_scalar(out=sgx2[:], in0=sgx[:], scalar1=2.0,
                                      scalar2=-1.0, op0=Alu.mult, op1=Alu.add)
        gxpi = t(); dve.tensor_tensor(out=gxpi[:], in0=a2[:], in1=sgx2[:],
                                      op=Alu.mult)
        gx = pt("gx"); dve.tensor_scalar(out=gx[:], in0=gxpi[:], scalar1=1.0 / PI,
                                    scalar2=None, op0=Alu.mult)

        # ---- gy: acos(y) = atan2(sqrt(1-y^2), y), same reduction ----
        yc = t(); dve.tensor_scalar(out=yc[:], in0=yt[:], scalar1=-1.0,
                                    scalar2=1.0, op0=Alu.max, op1=Alu.min)
        y2 = t(); act.activation(y2[:], yc[:], Act.Square, bias=cbias(0.0))
        sq = t(); act.activation(sq[:], y2[:], Act.Sqrt, bias=cbias(1.0),
                                 scale=-1.0)
        ay = t(); act.activation(ay[:], yc[:], Act.Abs, bias=cbias(0.0))
        mny = t(); dve.tensor_tensor(out=mny[:], in0=sq[:], in1=ay[:],
                                     op=Alu.min)
        mxy = t(); dve.tensor_tensor(out=mxy[:], in0=sq[:], in1=ay[:],
                                     op=Alu.max)
        dve.tensor_scalar(out=mxy[:], in0=mxy[:], scalar1=1e-38, scalar2=None,
                          op0=Alu.max)
        rmxy = t(); dve.reciprocal(rmxy[:], mxy[:])
        ry_ = t(); dve.tensor_tensor(out=ry_[:], in0=mny[:], in1=rmxy[:],
                                     op=Alu.mult)
        ac = t(); act.activation(ac[:], ry_[:], Act.Arctan, bias=cbias(0.0))
        swy = t(); dve.tensor_tensor(out=swy[:], in0=sq[:], in1=ay[:],
                                     op=Alu.is_gt)
        g1 = t(); dve.tensor_scalar(out=g1[:], in0=swy[:], scalar1=-2.0,
                                    scalar2=1.0, op0=Alu.mult, op1=Alu.add)
        ac1 = t(); dve.tensor_tensor(out=ac1[:], in0=ac[:], in1=g1[:],
                                     op=Alu.mult)
        dve.scalar_tensor_tensor(out=ac1[:], in0=swy[:], scalar=PI / 2,
                                 in1=ac1[:], op0=Alu.mult, op1=Alu.add)
        sylt = t(); dve.tensor_scalar(out=sylt[:], in0=yc[:], scalar1=0.0,
                                      scalar2=None, op0=Alu.is_lt)
        g2f = t(); dve.tensor_scalar(out=g2f[:], in0=sylt[:], scalar1=-2.0,
                                     scalar2=1.0, op0=Alu.mult, op1=Alu.add)
        ac2 = t(); dve.tensor_tensor(out=ac2[:], in0=ac1[:], in1=g2f[:],
                                     op=Alu.mult)
        dve.scalar_tensor_tensor(out=ac2[:], in0=sylt[:], scalar=PI,
                                 in1=ac2[:], op0=Alu.mult, op1=Alu.add)
        gy = pt("gy"); dve.tensor_scalar(out=gy[:], in0=ac2[:], scalar1=2.0 / PI,
                                    scalar2=-1.0, op0=Alu.mult, op1=Alu.add)

        # ---- mip weights: s0 = min(3-mip,1), s1 = clip(2-mip,0,1)/2 ----
        lnsa = t(); act.activation(lnsa[:], sat[:], Act.Ln, bias=cbias(0.0))
        mipc = t(); dve.tensor_scalar(out=mipc[:], in0=lnsa[:], scalar1=MIPC1,
                                      scalar2=MIPC2, op0=Alu.mult, op1=Alu.add)
        dve.tensor_scalar(out=mipc[:], in0=mipc[:], scalar1=1.0, scalar2=3.0,
                          op0=Alu.max, op1=Alu.min)
        neg = t(); dve.tensor_scalar(out=neg[:], in0=mipc[:], scalar1=-1.0,
                                     scalar2=3.0, op0=Alu.mult, op1=Alu.add)
        s0 = pt("s0"); dve.tensor_scalar(out=s0[:], in0=neg[:], scalar1=1.0,
                                    scalar2=None, op0=Alu.min)
        w1c = t(); dve.tensor_scalar(out=w1c[:, :CL], in0=neg[:, :CL],
                                     scalar1=1.0, scalar2=0.0,
                                     op0=Alu.subtract, op1=Alu.max)
        s1 = pt("s1"); dve.tensor_scalar(out=s1[:, :CL], in0=w1c[:, :CL],
                                    scalar1=1.0, scalar2=0.5, op0=Alu.min,
                                    op1=Alu.mult)
        slvl = [s0, s1]

        # ---- per-level coords, cubic weights, gather indices ----
        idxT = []
        w16 = geom.tile([P, NRP, 2, 16], f32, name="w16", tag="w16")
        for l, (H, W) in enumerate(dims):
            NC = NRP if l == 0 else CL      # level-1: only first CL columns
            ix4 = t(); act.activation(ix4[:, :NC], gx[:, :NC], Act.Identity,
                                      bias=cbias(W / 2.0 + 3.5), scale=W / 2.0)
            iy4 = t(); act.activation(iy4[:, :NC], gy[:, :NC], Act.Identity,
                                      bias=cbias(H / 2.0 + 3.5), scale=H / 2.0)

            def floorf(v4, NC=NC):
                vi = t(dt=i32); dve.tensor_copy(out=vi[:, :NC], in_=v4[:, :NC])
                vf = t(); dve.tensor_copy(out=vf[:, :NC], in_=vi[:, :NC])
                m = t(); dve.tensor_tensor(out=m[:, :NC], in0=vf[:, :NC],
                                           in1=v4[:, :NC], op=Alu.is_gt)
                vf2 = t(); dve.scalar_tensor_tensor(out=vf2[:, :NC],
                                                    in0=m[:, :NC],
                                                    scalar=-1.0,
                                                    in1=vf[:, :NC],
                                                    op0=Alu.mult, op1=Alu.add)
                return vf2

            xf = floorf(ix4)   # floor(ix) + 4
            yf = floorf(iy4)
            # entry idx first (gathers depend only on this, not the weights):
            # idx = (y0+1)*W + (x0-1) + 2 = yf*W + xf - 3*W - 3
            idf = t(); dve.scalar_tensor_tensor(out=idf[:, :NC],
                                                in0=yf[:, :NC],
                                                scalar=float(W),
                                                in1=xf[:, :NC],
                                                op0=Alu.mult, op1=Alu.add)
            dve.tensor_scalar(out=idf[:, :NC], in0=idf[:, :NC],
                              scalar1=float(-3 * W - 3), scalar2=None,
                              op0=Alu.add)
            idx = pt(f"idx{l}", dt=i32)
            dve.tensor_copy(out=idx[:, :NC], in_=idf[:, :NC])
            idxT.append(idx)

            tx = t(); dve.tensor_tensor(out=tx[:, :NC], in0=ix4[:, :NC],
                                        in1=xf[:, :NC], op=Alu.subtract)
            ty = t(); dve.tensor_tensor(out=ty[:, :NC], in0=iy4[:, :NC],
                                        in1=yf[:, :NC], op=Alu.subtract)

            # x tap k valid iff 0 <= (xf-4) - 1 + k <= W-1
            mks = geom.tile([P, NC, 4], f32, name=f"mks{l}", tag=f"mks{l}")
            dve.tensor_scalar(out=mks[:, :NC, 0], in0=xf[:, :NC], scalar1=4.5,
                              scalar2=None, op0=Alu.is_ge)
            dve.tensor_scalar(out=mks[:, :NC, 1], in0=xf[:, :NC], scalar1=3.5,
                              scalar2=None, op0=Alu.is_ge)
            dve.tensor_scalar(out=mks[:, :NC, 2], in0=xf[:, :NC],
                              scalar1=W + 2.5, scalar2=None, op0=Alu.is_le)
            dve.tensor_scalar(out=mks[:, :NC, 3], in0=xf[:, :NC],
                              scalar1=W + 1.5, scalar2=None, op0=Alu.is_le)

            wx = geom.tile([P, NC, 4], f32, name=f"wx{l}", tag=f"wx{l}")
            wy = geom.tile([P, NC, 4], f32, name=f"wy{l}", tag=f"wy{l}")
            for k, (d, c, b, a) in enumerate(CUBIC):
                p1 = t(); act.activation(p1[:, :NC], tx[:, :NC], Act.Identity,
                                         bias=cbias(c), scale=d)
                p2 = t(); dve.tensor_tensor(out=p2[:, :NC], in0=p1[:, :NC],
                                            in1=tx[:, :NC], op=Alu.mult)
                p3 = t(); dve.scalar_tensor_tensor(out=p3[:, :NC],
                                                   in0=p2[:, :NC],
                                                   scalar=b, in1=tx[:, :NC],
                                                   op0=Alu.add, op1=Alu.mult)
                dve.scalar_tensor_tensor(out=wx[:, :NC, k], in0=p3[:, :NC],
                                         scalar=a, in1=mks[:, :NC, k],
                                         op0=Alu.add, op1=Alu.mult)
                p1y = t(); act.activation(p1y[:, :NC], ty[:, :NC],
                                          Act.Identity, bias=cbias(c), scale=d)
                p2y = t(); dve.tensor_tensor(out=p2y[:, :NC], in0=p1y[:, :NC],
                                             in1=ty[:, :NC], op=Alu.mult)
                p3y = t(); dve.scalar_tensor_tensor(out=p3y[:, :NC],
                                                    in0=p2y[:, :NC],
                                                    scalar=b, in1=ty[:, :NC],
                                                    op0=Alu.add, op1=Alu.mult)
                dve.scalar_tensor_tensor(out=wy[:, :NC, k], in0=p3y[:, :NC],
                                         scalar=a, in1=slvl[l][:, :NC],
                                         op0=Alu.add, op1=Alu.mult)

            # W16[p, r, l, (k,j)] = wx_k * wy_j (mip weight folded in wy)
            dve.tensor_tensor(
                out=w16[:, :NC, l, :],
                in0=cap(wx, 0, [[NC * 4, P], [4, NC], [1, 4], [0, 4]]),
                in1=cap(wy, 0, [[NC * 4, P], [4, NC], [0, 4], [1, 4]]),
                op=Alu.mult)

        # ---- gather + weight + reduce + MLP ----
        emb = geom.tile([P, NRP, RANK], f32, name="emb", tag="emb")
        for ch in range(NCHUNK):
            g2 = gpool.tile([P, NRCH, 2, 16 * RANK], f32, name="g2", tag="g2")
            r0 = ch * NRCH
            for l in range(2):
                for rr in range(NRCH):
                    if l == 1 and r0 + rr >= L1C:
                        continue
                    nc.gpsimd.indirect_dma_start(
                        out=g2[:, rr, l, :],
                        out_offset=None,
                        in_=ex[l][:, :],
                        in_offset=bass.IndirectOffsetOnAxis(
                            ap=idxT[l][:, r0 + rr:r0 + rr + 1], axis=0))
            # column segments: ntap=32 where level-1 was gathered, else 16
            if r0 + NRCH <= L1C:
                segs = [(0, NRCH, 32)]
            elif r0 >= L1C:
                segs = [(0, NRCH, 16)]
            else:
                segs = [(0, L1C - r0, 32), (L1C - r0, NRCH, 16)]
            for (sa_, sb_, ntap) in segs:
                gflat = cap(g2, sa_ * 256,
                            [[NRCH * 256, P], [256, sb_ - sa_], [8, ntap],
                             [1, 8]])
                dve.tensor_tensor(
                    out=gflat, in0=gflat,
                    in1=cap(w16, (r0 + sa_) * 32,
                            [[NRP * 32, P], [32, sb_ - sa_], [1, ntap],
                             [0, 8]]),
                    op=Alu.mult)
                dve.tensor_reduce(
                    out=emb[:, r0 + sa_:r0 + sb_, :],
                    in_=cap(g2, sa_ * 256,
                            [[NRCH * 256, P], [256, sb_ - sa_], [1, 8],
                             [8, ntap]]),
                    axis=mybir.AxisListType.X, op=Alu.add)

            for g in range(ch * (NRCH // GR), (ch + 1) * (NRCH // GR)):
                embT_ps = pp.tile([P, P], f32, name="pT", tag="pT")
                nc.tensor.transpose(
                    out=embT_ps[:],
                    in_=cap(emb, g * GR * RANK, [[NRP * RANK, P],
                                                 [1, GR * RANK]]),
                    identity=ident[:])
                embT = mpool.tile([P, P], f32, name="embT", tag="embT")
                act.copy(embT[:], embT_ps[:])
                outsb = mpool.tile([3, GR * P], f32, name="outsb", tag="outsb")
                for half in range(GR // 4):
                    h_ps = pp.tile([P, 4 * P], f32, name="hps", tag="h")
                    for s in range(4):
                        rb = half * 4 + s
                        nc.tensor.matmul(
                            out=h_ps[:, s * P:(s + 1) * P],
                            lhsT=w1_sb[:, rb * P:(rb + 1) * P],
                            rhs=embT[:],
                            start=True, stop=True)
                    h_sb = mpool.tile([P, 4 * P], f32, name="hsb", tag="hsb")
                    act.activation(h_sb[:], h_ps[:], Act.Relu, bias=cbias(0.0))
                    o_ps = pp.tile([3, 4 * P], f32, name="ops", tag="o")
                    nc.tensor.matmul(out=o_ps[:], lhsT=w2_sb[:], rhs=h_sb[:],
                                     start=True, stop=True)
                    act.copy(outsb[:, half * 4 * P:(half + 1) * 4 * P],
                             o_ps[:])
                nc.sync.dma_start(out_d[g], outsb[:])

    nc.compile()
    return nc


_NC_CACHE = {}


def get_nc(key, cfg):
    if key not in _NC_CACHE:
        _NC_CACHE[key] = build_nc(cfg)
    return _NC_CACHE[key]


def _host_geom(vd, cfg):
    """Replicate device index math (float64): level-0 entry idx per ray."""
    H, W = cfg["dims"][0]
    x, y, z = vd[:, 0].astype(np.float64), vd[:, 1].astype(np.float64), \
        vd[:, 2].astype(np.float64)
    gx = np.arctan2(x, z) / np.pi
    gy = np.arccos(np.clip(y, -1.0, 1.0)) / np.pi * 2.0 - 1.0
    xf = np.floor((gx + 1.0) * W / 2.0 + 3.5)
    yf = np.floor((gy + 1.0) * H / 2.0 + 3.5)
    return (yf * W + xf - 3 * W - 3).astype(np.int64)


def host_prepare(viewdirs, saSample, mats, W1, W2, cfg):
    nrc = cfg["nrc"]
    ND = cfg["num_devices"]
    NRP = nrc // P
    vd = np.asarray(viewdirs, np.float32)
    sa = np.asarray(saSample, np.float32)

    # sort: level-1-needing rays first (by sa ascending, so any overflow
    # hits s1 -> 0 rays), then the rest by level-0 table position (gather
    # locality)
    saTexel = 4.0 * PI / (6.0 * cfg["res_mip"] ** 2)
    needs = sa < np.float32(16.0 * saTexel) * np.float32(1.0000005)
    idx0 = _host_geom(vd, cfg)
    i_l1 = np.where(needs)[0]
    i_l1 = i_l1[np.argsort(sa[i_l1], kind="stable")]
    i_rest = np.where(~needs)[0]
    i_rest = i_rest[np.argsort(idx0[i_rest], kind="stable")]
    S = np.concatenate([i_l1, i_rest])

    cap_l1 = cfg["l1_cols"] * P
    assert (len(i_l1) + ND - 1) // ND <= cap_l1, (
        f"level-1 rays per core {(len(i_l1)+ND-1)//ND} exceed capacity "
        f"{cap_l1}; raise l1_cols")

    ex_tabs = [_expand_table(m) for m in mats]
    w1big = np.zeros((P, 16 * P), np.float32)
    w1t = np.asarray(W1, np.float32).T    # [8, 128]
    for rb in range(16):
        w1big[rb * 8:(rb + 1) * 8, rb * P:(rb + 1) * P] = w1t
    w2t = np.ascontiguousarray(np.asarray(W2, np.float32).T)
    in_maps, grids = [], []
    for c in range(ND):
        Sc = S[c::ND]
        idxg = Sc.reshape(NRP, P).T       # [P, NRP]; column-major fill
        m = {"vdT": np.ascontiguousarray(vd[idxg].transpose(2, 0, 1)),
             "sa": np.ascontiguousarray(sa[idxg]),
             "w1big": w1big, "w2t": w2t}
        for l, tab in enumerate(ex_tabs):
            m[f"ex{l}"] = tab
        in_maps.append(m)
        grids.append(idxg)
    return in_maps, grids


def assemble_output(results, grids, cfg):
    nrc = cfg["nrc"]
    NRP = nrc // P
    GR = min(16, NRP)
    full = np.empty((nrc * cfg["num_devices"], 3), np.float32)
    for res, idxg in zip(results, grids):
        o = res["out"]                     # [ngrp, 3, GR*P]
        ngrp = o.shape[0]
        o = o.reshape(ngrp, 3, GR, P)
        # core ray at (p, col=g*GR+b) -> o[g, :, b, p]
        core = o.transpose(3, 0, 2, 1).reshape(P, NRP, 3)
        full[idxg.reshape(-1)] = core.reshape(-1, 3)
    return full


def kernel(viewdirs, saSample, bg_mat0, bg_mat1, bg_mat2, W1, W2):
    from concourse.bass_utils import run_bass_kernel_spmd
    cfg = FULL_CFG
    nc = get_nc("full", cfg)
    in_maps, grids = host_prepare(viewdirs, saSample, [bg_mat0, bg_mat1],
                                  W1, W2, cfg)
    res = run_bass_kernel_spmd(nc, in_maps, list(range(cfg["num_devices"])))
    return assemble_output(res.results, grids, cfg)


# revision 29
# speedup vs baseline: 1.0155x; 1.0155x over previous
"""Trainium2 Bass kernel for HierarchicalBG embedding lookup (bicubic
pano-grid sampling + tiny MLP), data-parallel over rays on 8 NeuronCores.

Key facts exploited:
- Level-2 grid weight clip(1-mip,0,1) == 0 identically (mip >= 1): the 256MB
  bg_mat2 is never read.
- Level-1 weight clip(2-mip,0,1) == 0 for ~82% of rays (a pure function of
  saSample, known on host). Host sorts level-1-needing rays into the first
  l1_chunks chunks of each core; level-1 gathers/weighting are only issued
  there (graceful degradation: any overflow hits s1 -> 0 rays).
- Remaining rays are sorted by level-0 table position so consecutive
  gathers hit nearby HBM rows.
- Grids are re-laid out on host into a 4-y-tap expanded table so one ray's
  4x4x8 bicubic footprint is one contiguous 512B run -> one indirect-DMA
  descriptor per (ray, level).
- Per-ray tap weights (wx x wy outer product, mip weight folded in) applied
  on DVE in ray-major layout; tap reduction on DVE; 8->128->3 MLP on PE after
  a 128x128 PE transpose.
"""

import numpy as np
from contextlib import ExitStack

PI = float(np.pi)
RANK = 8
P = 128

# cubic weights as polys in t = frac coordinate, coeffs (d, c, b, a) for
# w = ((d*t + c)*t + b)*t + a   (A = -0.75, matches reference _cubic_weights)
CUBIC = [
    (-0.75, 1.50, -0.75, 0.0),   # c2(1+t)
    (1.25, -2.25, 0.0, 1.0),     # c1(t)
    (-1.25, 1.50, 0.75, 0.0),    # c1(1-t)
    (0.75, -0.75, 0.0, 0.0),     # c2(2-t)
]

FULL_CFG = dict(
    nrc=32768,
    dims=((512, 1024), (1024, 2048)),
    res_mip=2048,
    nr_chunk=32,
    l1_chunks=2,
    l1_cols=48,
    num_devices=8,
)


def _expand_table(img):
    """[C, H, W] -> [(H+1)*W + 8, 32]: entry (R, x) = rows R-2..R+1 at col x,
    channels innermost, zero outside the image; +2 entry global pad."""
    C, H, W = img.shape
    imgT = np.ascontiguousarray(np.asarray(img, np.float32).transpose(1, 2, 0))
    ex = np.zeros((H + 1, W, 4, C), np.float32)
    for j in range(4):
        lo = max(0, 2 - j)
        hi = min(H, H + 1 - j)
        ex[lo:hi + 1, :, j, :] = imgT[lo - 2 + j:hi - 1 + j, :, :]
    flat = np.zeros(((H + 1) * W + 8, 4 * C), np.float32)
    flat[2:2 + (H + 1) * W] = ex.reshape(-1, 4 * C)
    return flat


def build_nc(cfg):
    import concourse.bass as bass
    import concourse.tile as tile
    from concourse import bacc, mybir

    f32 = mybir.dt.float32
    i32 = mybir.dt.int32
    Alu = mybir.AluOpType
    Act = mybir.ActivationFunctionType

    nrc, dims, NRCH = cfg["nrc"], cfg["dims"], cfg["nr_chunk"]
    NRP = nrc // P
    NCHUNK = NRP // NRCH
    L1CH = cfg["l1_chunks"]
    CL = L1CH * NRCH                       # columns with level-1 geometry
    L1C = cfg["l1_cols"]                   # columns with level-1 gathers
    GR = min(16, NRP)
    n_ent = [(h + 1) * w + 8 for (h, w) in dims]
    saTexel = 4.0 * PI / (6.0 * cfg["res_mip"] ** 2)
    MIPC1 = 1.0 / (2.0 * np.log(2.0))
    MIPC2 = -float(np.log(saTexel)) * MIPC1

    nc = bacc.Bacc("TRN2", target_bir_lowering=False, debug=False,
                   num_devices=cfg["num_devices"])
    vd = nc.dram_tensor("vdT", [3, P, NRP], f32, kind="ExternalInput").ap()
    sa = nc.dram_tensor("sa", [P, NRP], f32, kind="ExternalInput").ap()
    ex = [nc.dram_tensor(f"ex{l}", [n_ent[l], 4 * RANK], f32,
                         kind="ExternalInput").ap() for l in range(2)]
    w1r = nc.dram_tensor("w1big", [P, 16 * P], f32, kind="ExternalInput").ap()
    w2t = nc.dram_tensor("w2t", [P, 3], f32, kind="ExternalInput").ap()
    out_d = nc.dram_tensor("out", [NRP // GR, 3, GR * P], f32,
                           kind="ExternalOutput").ap()

    def cap(tile_obj, offset, ap_list):
        """custom AP over a tile's underlying tensor (element units)."""
        base = tile_obj[:]
        return bass.AP(base.tensor, base.offset + offset, ap_list)

    with tile.TileContext(nc) as tc, ExitStack() as ctx:
        from concourse.masks import make_identity
        cpool = ctx.enter_context(tc.tile_pool(name="const", bufs=1))
        geom = ctx.enter_context(tc.tile_pool(name="geom", bufs=1))
        gpool = ctx.enter_context(tc.tile_pool(name="gath", bufs=2))
        mpool = ctx.enter_context(tc.tile_pool(name="mlp", bufs=2))
        pp = ctx.enter_context(tc.tile_pool(name="ps", bufs=2, space="PSUM"))
        tpool = ctx.enter_context(tc.tile_pool(name="tmp", bufs=12))
        dve, act = nc.vector, nc.scalar

        _tag = [0]

        def t(shape=None, dt=f32):
            _tag[0] += 1
            return tpool.tile([P, NRP] if shape is None else shape, dt,
                              name=f"g{_tag[0]}", tag="tmp")

        def pt(nm, shape=None, dt=f32):
            return geom.tile([P, NRP] if shape is None else shape, dt,
                             name=nm, tag=nm)

        _cb = {}

        def cbias(val):
            if val not in _cb:
                ct = cpool.tile([P, 1], f32, name=f"cb{len(_cb)}",
                                tag=f"cb{len(_cb)}")
                nc.vector.memset(ct[:], float(val))
                _cb[val] = ct
            return _cb[val][:]

        ident = cpool.tile([P, P], f32, name="ident", tag="ident")
        make_identity(nc, ident[:])
        w1_sb = cpool.tile([P, 16 * P], f32, name="w1c_", tag="w1")
        nc.sync.dma_start(w1_sb[:], w1r[:, :])
        w2_sb = cpool.tile([P, 3], f32, name="w2c_", tag="w2")
        nc.sync.dma_start(w2_sb[:], w2t[:, :])

        xt, yt, zt, sat = pt("xt"), pt("yt"), pt("zt"), pt("sat")
        nc.sync.dma_start(xt[:], vd[0])
        nc.sync.dma_start(yt[:], vd[1])
        nc.sync.dma_start(zt[:], vd[2])
        nc.sync.dma_start(sat[:], sa[:, :])

        # ---- gx*pi = atan2(x,z) via range-reduced arctan ----
        ax = t(); act.activation(ax[:], xt[:], Act.Abs, bias=cbias(0.0))
        az = t(); act.activation(az[:], zt[:], Act.Abs, bias=cbias(0.0))
        mn = t(); dve.tensor_tensor(out=mn[:], in0=ax[:], in1=az[:], op=Alu.min)
        mx = t(); dve.tensor_tensor(out=mx[:], in0=ax[:], in1=az[:], op=Alu.max)
        dve.tensor_scalar(out=mx[:], in0=mx[:], scalar1=1e-38, scalar2=None,
                          op0=Alu.max)
        rmx = t(); dve.reciprocal(rmx[:], mx[:])
        r = t(); dve.tensor_tensor(out=r[:], in0=mn[:], in1=rmx[:], op=Alu.mult)
        a = t(); act.activation(a[:], r[:], Act.Arctan, bias=cbias(0.0))
        swp = t(); dve.tensor_tensor(out=swp[:], in0=ax[:], in1=az[:],
                                     op=Alu.is_gt)
        f1 = t(); dve.tensor_scalar(out=f1[:], in0=swp[:], scalar1=-2.0,
                                    scalar2=1.0, op0=Alu.mult, op1=Alu.add)
        a1 = t(); dve.tensor_tensor(out=a1[:], in0=a[:], in1=f1[:], op=Alu.mult)
        dve.scalar_tensor_tensor(out=a1[:], in0=swp[:], scalar=PI / 2,
                                 in1=a1[:], op0=Alu.mult, op1=Alu.add)
        szlt = t(); dve.tensor_scalar(out=szlt[:], in0=zt[:], scalar1=0.0,
                                      scalar2=None, op0=Alu.is_lt)
        f2 = t(); dve.tensor_scalar(out=f2[:], in0=szlt[:], scalar1=-2.0,
                                    scalar2=1.0, op0=Alu.mult, op1=Alu.add)
        a2 = t(); dve.tensor_tensor(out=a2[:], in0=a1[:], in1=f2[:],
                                    op=Alu.mult)
        dve.scalar_tensor_tensor(out=a2[:], in0=szlt[:], scalar=PI,
                                 in1=a2[:], op0=Alu.mult, op1=Alu.add)
        sgx = t(); dve.tensor_scalar(out=sgx[:], in0=xt[:], scalar1=0.0,
                                     scalar2=None, op0=Alu.is_ge)
        sgx2 = t(); dve.tensor_scalar(out=sgx2[:], in0=sgx[:], scalar1=2.0,
                                      scalar2=-1.0, op0=Alu.mult, op1=Alu.add)
        gxpi = t(); dve.tensor_tensor(out=gxpi[:], in0=a2[:], in1=sgx2[:],
                                      op=Alu.mult)
        gx = pt("gx"); dve.tensor_scalar(out=gx[:], in0=gxpi[:], scalar1=1.0 / PI,
                                    scalar2=None, op0=Alu.mult)

        # ---- gy: acos(y) = atan2(sqrt(1-y^2), y), same reduction ----
        yc = t(); dve.tensor_scalar(out=yc[:], in0=yt[:], scalar1=-1.0,
                                    scalar2=1.0, op0=Alu.max, op1=Alu.min)
        y2 = t(); act.activation(y2[:], yc[:], Act.Square, bias=cbias(0.0))
        sq = t(); act.activation(sq[:], y2[:], Act.Sqrt, bias=cbias(1.0),
                                 scale=-1.0)
        ay = t(); act.activation(ay[:], yc[:], Act.Abs, bias=cbias(0.0))
        mny = t(); dve.tensor_tensor(out=mny[:], in0=sq[:], in1=ay[:],
                                     op=Alu.min)
        mxy = t(); dve.tensor_tensor(out=mxy[:], in0=sq[:], in1=ay[:],
                                     op=Alu.max)
        dve.tensor_scalar(out=mxy[:], in0=mxy[:], scalar1=1e-38, scalar2=None,
                          op0=Alu.max)
        rmxy = t(); dve.reciprocal(rmxy[:], mxy[:])
        ry_ = t(); dve.tensor_tensor(out=ry_[:], in0=mny[:], in1=rmxy[:],
                                     op=Alu.mult)
        ac = t(); act.activation(ac[:], ry_[:], Act.Arctan, bias=cbias(0.0))
        swy = t(); dve.tensor_tensor(out=swy[:], in0=sq[:], in1=ay[:],
                                     op=Alu.is_gt)
        g1 = t(); dve.tensor_scalar(out=g1[:], in0=swy[:], scalar1=-2.0,
                                    scalar2=1.0, op0=Alu.mult, op1=Alu.add)
        ac1 = t(); dve.tensor_tensor(out=ac1[:], in0=ac[:], in1=g1[:],
                                     op=Alu.mult)
        dve.scalar_tensor_tensor(out=ac1[:], in0=swy[:], scalar=PI / 2,
                                 in1=ac1[:], op0=Alu.mult, op1=Alu.add)
        sylt = t(); dve.tensor_scalar(out=sylt[:], in0=yc[:], scalar1=0.0,
                                      scalar2=None, op0=Alu.is_lt)
        g2f = t(); dve.tensor_scalar(out=g2f[:], in0=sylt[:], scalar1=-2.0,
                                     scalar2=1.0, op0=Alu.mult, op1=Alu.add)
        ac2 = t(); dve.tensor_tensor(out=ac2[:], in0=ac1[:], in1=g2f[:],
                                     op=Alu.mult)
        dve.scalar_tensor_tensor(out=ac2[:], in0=sylt[:], scalar=PI,
                                 in1=ac2[:], op0=Alu.mult, op1=Alu.add)
        gy = pt("gy"); dve.tensor_scalar(out=gy[:], in0=ac2[:], scalar1=2.0 / PI,
                                    scalar2=-1.0, op0=Alu.mult, op1=Alu.add)

        # ---- per-level coords, cubic weights, gather indices ----
        slvl = []
        idxT = []
        w16 = geom.tile([P, NRP, 2, 16], f32, name="w16", tag="w16")
        for l, (H, W) in enumerate(dims):
            NC = NRP if l == 0 else CL      # level-1: only first CL columns
            ix4 = t(); act.activation(ix4[:, :NC], gx[:, :NC], Act.Identity,
                                      bias=cbias(W / 2.0 + 3.5), scale=W / 2.0)
            iy4 = t(); act.activation(iy4[:, :NC], gy[:, :NC], Act.Identity,
                                      bias=cbias(H / 2.0 + 3.5), scale=H / 2.0)

            def floorf(v4, NC=NC):
                vi = t(dt=i32); dve.tensor_copy(out=vi[:, :NC], in_=v4[:, :NC])
                vf = t(); dve.tensor_copy(out=vf[:, :NC], in_=vi[:, :NC])
                m = t(); dve.tensor_tensor(out=m[:, :NC], in0=vf[:, :NC],
                                           in1=v4[:, :NC], op=Alu.is_gt)
                vf2 = t(); dve.scalar_tensor_tensor(out=vf2[:, :NC],
                                                    in0=m[:, :NC],
                                                    scalar=-1.0,
                                                    in1=vf[:, :NC],
                                                    op0=Alu.mult, op1=Alu.add)
                return vf2

            xf = floorf(ix4)   # floor(ix) + 4
            yf = floorf(iy4)
            # entry idx first (gathers depend only on this, not the weights):
            # idx = (y0+1)*W + (x0-1) + 2 = yf*W + xf - 3*W - 3
            idf = t(); dve.scalar_tensor_tensor(out=idf[:, :NC],
                                                in0=yf[:, :NC],
                                                scalar=float(W),
                                                in1=xf[:, :NC],
                                                op0=Alu.mult, op1=Alu.add)
            dve.tensor_scalar(out=idf[:, :NC], in0=idf[:, :NC],
                              scalar1=float(-3 * W - 3), scalar2=None,
                              op0=Alu.add)
            idx = pt(f"idx{l}", dt=i32)
            dve.tensor_copy(out=idx[:, :NC], in_=idf[:, :NC])
            idxT.append(idx)

            if l == 0:
                # mip weights off the idx critical path:
                # s0 = min(3-mip,1), s1 = clip(2-mip,0,1)/2
                lnsa = t(); act.activation(lnsa[:], sat[:], Act.Ln,
                                           bias=cbias(0.0))
                mipc = t(); dve.tensor_scalar(out=mipc[:], in0=lnsa[:],
                                              scalar1=MIPC1, scalar2=MIPC2,
                                              op0=Alu.mult, op1=Alu.add)
                dve.tensor_scalar(out=mipc[:], in0=mipc[:], scalar1=1.0,
                                  scalar2=3.0, op0=Alu.max, op1=Alu.min)
                neg = t(); dve.tensor_scalar(out=neg[:], in0=mipc[:],
                                             scalar1=-1.0, scalar2=3.0,
                                             op0=Alu.mult, op1=Alu.add)
                s0 = pt("s0"); dve.tensor_scalar(out=s0[:], in0=neg[:],
                                            scalar1=1.0, scalar2=None,
                                            op0=Alu.min)
                w1c = t(); dve.tensor_scalar(out=w1c[:, :CL], in0=neg[:, :CL],
                                             scalar1=1.0, scalar2=0.0,
                                             op0=Alu.subtract, op1=Alu.max)
                s1 = pt("s1"); dve.tensor_scalar(out=s1[:, :CL],
                                            in0=w1c[:, :CL], scalar1=1.0,
                                            scalar2=0.5, op0=Alu.min,
                                            op1=Alu.mult)
                slvl.extend([s0, s1])

            tx = t(); dve.tensor_tensor(out=tx[:, :NC], in0=ix4[:, :NC],
                                        in1=xf[:, :NC], op=Alu.subtract)
            ty = t(); dve.tensor_tensor(out=ty[:, :NC], in0=iy4[:, :NC],
                                        in1=yf[:, :NC], op=Alu.subtract)

            # x tap k valid iff 0 <= (xf-4) - 1 + k <= W-1
            mks = geom.tile([P, NC, 4], f32, name=f"mks{l}", tag=f"mks{l}")
            dve.tensor_scalar(out=mks[:, :NC, 0], in0=xf[:, :NC], scalar1=4.5,
                              scalar2=None, op0=Alu.is_ge)
            dve.tensor_scalar(out=mks[:, :NC, 1], in0=xf[:, :NC], scalar1=3.5,
                              scalar2=None, op0=Alu.is_ge)
            dve.tensor_scalar(out=mks[:, :NC, 2], in0=xf[:, :NC],
                              scalar1=W + 2.5, scalar2=None, op0=Alu.is_le)
            dve.tensor_scalar(out=mks[:, :NC, 3], in0=xf[:, :NC],
                              scalar1=W + 1.5, scalar2=None, op0=Alu.is_le)

            wx = geom.tile([P, NC, 4], f32, name=f"wx{l}", tag=f"wx{l}")
            wy = geom.tile([P, NC, 4], f32, name=f"wy{l}", tag=f"wy{l}")
            for k, (d, c, b, a) in enumerate(CUBIC):
                p1 = t(); act.activation(p1[:, :NC], tx[:, :NC], Act.Identity,
                                         bias=cbias(c), scale=d)
                p2 = t(); dve.tensor_tensor(out=p2[:, :NC], in0=p1[:, :NC],
                                            in1=tx[:, :NC], op=Alu.mult)
                p3 = t(); dve.scalar_tensor_tensor(out=p3[:, :NC],
                                                   in0=p2[:, :NC],
                                                   scalar=b, in1=tx[:, :NC],
                                                   op0=Alu.add, op1=Alu.mult)
                dve.scalar_tensor_tensor(out=wx[:, :NC, k], in0=p3[:, :NC],
                                         scalar=a, in1=mks[:, :NC, k],
                                         op0=Alu.add, op1=Alu.mult)
                p1y = t(); act.activation(p1y[:, :NC], ty[:, :NC],
                                          Act.Identity, bias=cbias(c), scale=d)
                p2y = t(); dve.tensor_tensor(out=p2y[:, :NC], in0=p1y[:, :NC],
                                             in1=ty[:, :NC], op=Alu.mult)
                p3y = t(); dve.scalar_tensor_tensor(out=p3y[:, :NC],
                                                    in0=p2y[:, :NC],
                                                    scalar=b, in1=ty[:, :NC],
                                                    op0=Alu.add, op1=Alu.mult)
                dve.scalar_tensor_tensor(out=wy[:, :NC, k], in0=p3y[:, :NC],
                                         scalar=a, in1=slvl[l][:, :NC],
                                         op0=Alu.add, op1=Alu.mult)

            # W16[p, r, l, (k,j)] = wx_k * wy_j (mip weight folded in wy)
            dve.tensor_tensor(
                out=w16[:, :NC, l, :],
                in0=cap(wx, 0, [[NC * 4, P], [4, NC], [1, 4], [0, 4]]),
                in1=cap(wy, 0, [[NC * 4, P], [4, NC], [0, 4], [1, 4]]),
                op=Alu.mult)

        # ---- gather + weight + reduce + MLP ----
        # last chunk split in two: halves the post-last-gather tail
        chunk_list = [(k * NRCH, NRCH) for k in range(NCHUNK - 1)]
        chunk_list += [((NCHUNK - 1) * NRCH, NRCH // 2),
                       ((NCHUNK - 1) * NRCH + NRCH // 2, NRCH // 2)]
        emb = geom.tile([P, NRP, RANK], f32, name="emb", tag="emb")
        for (r0, ncols) in chunk_list:
            g2 = gpool.tile([P, NRCH, 2, 16 * RANK], f32, name="g2", tag="g2")
            for l in range(2):
                for rr in range(ncols):
                    if l == 1 and r0 + rr >= L1C:
                        continue
                    nc.gpsimd.indirect_dma_start(
                        out=g2[:, rr, l, :],
                        out_offset=None,
                        in_=ex[l][:, :],
                        in_offset=bass.IndirectOffsetOnAxis(
                            ap=idxT[l][:, r0 + rr:r0 + rr + 1], axis=0))
            # column segments: ntap=32 where level-1 was gathered, else 16
            if r0 + ncols <= L1C:
                segs = [(0, ncols, 32)]
            elif r0 >= L1C:
                segs = [(0, ncols, 16)]
            else:
                segs = [(0, L1C - r0, 32), (L1C - r0, ncols, 16)]
            for (sa_, sb_, ntap) in segs:
                gflat = cap(g2, sa_ * 256,
                            [[NRCH * 256, P], [256, sb_ - sa_], [8, ntap],
                             [1, 8]])
                dve.tensor_tensor(
                    out=gflat, in0=gflat,
                    in1=cap(w16, (r0 + sa_) * 32,
                            [[NRP * 32, P], [32, sb_ - sa_], [1, ntap],
                             [0, 8]]),
                    op=Alu.mult)
                dve.tensor_reduce(
                    out=emb[:, r0 + sa_:r0 + sb_, :],
                    in_=cap(g2, sa_ * 256,
                            [[NRCH * 256, P], [256, sb_ - sa_], [1, 8],
                             [8, ntap]]),
                    axis=mybir.AxisListType.X, op=Alu.add)

            for g in range(r0 // GR, (r0 + ncols) // GR):
                embT_ps = pp.tile([P, P], f32, name="pT", tag="pT")
                nc.tensor.transpose(
                    out=embT_ps[:],
                    in_=cap(emb, g * GR * RANK, [[NRP * RANK, P],
                                                 [1, GR * RANK]]),
                    identity=ident[:])
                embT = mpool.tile([P, P], f32, name="embT", tag="embT")
                act.copy(embT[:], embT_ps[:])
                outsb = mpool.tile([3, GR * P], f32, name="outsb", tag="outsb")
                for half in range(GR // 4):
                    h_ps = pp.tile([P, 4 * P], f32, name="hps", tag="h")
                    for s in range(4):
                        rb = half * 4 + s
                        nc.tensor.matmul(
                            out=h_ps[:, s * P:(s + 1) * P],
                            lhsT=w1_sb[:, rb * P:(rb + 1) * P],
                            rhs=embT[:],
                            start=True, stop=True)
                    h_sb = mpool.tile([P, 4 * P], f32, name="hsb", tag="hsb")
                    act.activation(h_sb[:], h_ps[:], Act.Relu, bias=cbias(0.0))
                    o_ps = pp.tile([3, 4 * P], f32, name="ops", tag="o")
                    nc.tensor.matmul(out=o_ps[:], lhsT=w2_sb[:], rhs=h_sb[:],
                                     start=True, stop=True)
                    act.copy(outsb[:, half * 4 * P:(half + 1) * 4 * P],
                             o_ps[:])
                nc.sync.dma_start(out_d[g], outsb[:])

    nc.compile()
    return nc


_NC_CACHE = {}


def get_nc(key, cfg):
    if key not in _NC_CACHE:
        _NC_CACHE[key] = build_nc(cfg)
    return _NC_CACHE[key]


def _host_geom(vd, cfg):
    """Replicate device index math (float64): level-0 entry idx per ray."""
    H, W = cfg["dims"][0]
    x, y, z = vd[:, 0].astype(np.float64), vd[:, 1].astype(np.float64), \
        vd[:, 2].astype(np.float64)
    gx = np.arctan2(x, z) / np.pi
    gy = np.arccos(np.clip(y, -1.0, 1.0)) / np.pi * 2.0 - 1.0
    xf = np.floor((gx + 1.0) * W / 2.0 + 3.5)
    yf = np.floor((gy + 1.0) * H / 2.0 + 3.5)
    return (yf * W + xf - 3 * W - 3).astype(np.int64)


def host_prepare(viewdirs, saSample, mats, W1, W2, cfg):
    nrc = cfg["nrc"]
    ND = cfg["num_devices"]
    NRP = nrc // P
    vd = np.asarray(viewdirs, np.float32)
    sa = np.asarray(saSample, np.float32)

    # sort: level-1-needing rays first (by sa ascending, so any overflow
    # hits s1 -> 0 rays), then the rest by level-0 table position (gather
    # locality)
    saTexel = 4.0 * PI / (6.0 * cfg["res_mip"] ** 2)
    needs = sa < np.float32(16.0 * saTexel) * np.float32(1.0000005)
    idx0 = _host_geom(vd, cfg)
    i_l1 = np.where(needs)[0]
    i_l1 = i_l1[np.argsort(sa[i_l1], kind="stable")]
    i_rest = np.where(~needs)[0]
    i_rest = i_rest[np.argsort(idx0[i_rest], kind="stable")]
    S = np.concatenate([i_l1, i_rest])

    cap_l1 = cfg["l1_cols"] * P
    assert (len(i_l1) + ND - 1) // ND <= cap_l1, (
        f"level-1 rays per core {(len(i_l1)+ND-1)//ND} exceed capacity "
        f"{cap_l1}; raise l1_cols")

    ex_tabs = [_expand_table(m) for m in mats]
    w1big = np.zeros((P, 16 * P), np.float32)
    w1t = np.asarray(W1, np.float32).T    # [8, 128]
    for rb in range(16):
        w1big[rb * 8:(rb + 1) * 8, rb * P:(rb + 1) * P] = w1t
    w2t = np.ascontiguousarray(np.asarray(W2, np.float32).T)
    in_maps, grids = [], []
    for c in range(ND):
        Sc = S[c::ND]
        idxg = Sc.reshape(NRP, P).T       # [P, NRP]; column-major fill
        m = {"vdT": np.ascontiguousarray(vd[idxg].transpose(2, 0, 1)),
             "sa": np.ascontiguousarray(sa[idxg]),
             "w1big": w1big, "w2t": w2t}
        for l, tab in enumerate(ex_tabs):
            m[f"ex{l}"] = tab
        in_maps.append(m)
        grids.append(idxg)
    return in_maps, grids


def assemble_output(results, grids, cfg):
    nrc = cfg["nrc"]
    NRP = nrc // P
    GR = min(16, NRP)
    full = np.empty((nrc * cfg["num_devices"], 3), np.float32)
    for res, idxg in zip(results, grids):
        o = res["out"]                     # [ngrp, 3, GR*P]
        ngrp = o.shape[0]
        o = o.reshape(ngrp, 3, GR, P)
        # core ray at (p, col=g*GR+b) -> o[g, :, b, p]
        core = o.transpose(3, 0, 2, 1).reshape(P, NRP, 3)
        full[idxg.reshape(-1)] = core.reshape(-1, 3)
    return full


def kernel(viewdirs, saSample, bg_mat0, bg_mat1, bg_mat2, W1, W2):
    from concourse.bass_utils import run_bass_kernel_spmd
    cfg = FULL_CFG
    nc = get_nc("full", cfg)
    in_maps, grids = host_prepare(viewdirs, saSample, [bg_mat0, bg_mat1],
                                  W1, W2, cfg)
    res = run_bass_kernel_spmd(nc, in_maps, list(range(cfg["num_devices"])))
    return assemble_output(res.results, grids, cfg)
